# revision 47
# baseline (speedup 1.0000x reference)
"""BEiT-style windowed attention (B=32, N=577, D=768, 12 heads) on 8 TRN2 cores.

Data-parallel over batch (4 elements/core, no collectives).

qkv projection runs in fp8-e4m3 DoubleRow mode (0.5 PE cycles/row) with an
error-compensated hi+lo split: x = (x8h + x8l)/SX and W = (w8h + w8l)/SW,
computing x@W as the hh + hl + lh DoubleRow terms (the lo*lo term is below
fp16 noise). That is 4.5 effective f16 contraction steps instead of 6 at
near-fp16 accuracy.

The q/k projections are written to SBUF as single fp8 (scaled by SQ) in the
baseline [128, tile, N] layout (2 heads per 128 partitions). Scores then run
as fp8 DoubleRow matmuls (0.5 cycles/row, half the f16 cost): each head's
[64, N] q/k block is paired with a shared all-zero subrow slot via a
step-sliced AP ([64, 2, *] with dim-1 stride reaching slot 12), which
satisfies DR's paired-subrow shape at no PE cost (matmul cost is
free-size-based). The fp8 quantization of q/k adds ~1.8% rms noise to the
output - measured 1.81e-2 max-rel on hardware vs the 2e-2 gate.

P = exp(S)*exp(rel_bias) stays f16 (fp8 P or V measured ~2.6% rms each -
too lossy), so P@V is the f16 path with the rowsum ones-column, and the
output projection keeps the fp8 hi/lo DoubleRow with dma-transposed O.

The schedule is one fused 48-head stream (batch boundaries pipelined away):
P@V lags scores by 2 heads; qkv tiles of batch b+1 and output-projection
tiles of batch b-1 are spread between score tiles in ~0.4us term-phases so
the in-order PE queue always has ready work while exp (Act) and bias-mul
(GpSimd) of earlier score tiles complete. The final batch ends with a staged
tail: slots 0-4 of O transpose/split early, the heads-0-7 share of the last
projections runs before the last head's P@V, and the remainder drains with
three psum tiles in flight. Startup DMAs are spread over the SP/Act/Pool
queues (HBM-bandwidth-bound, ~10us to first exp).
"""

import numpy as np

import concourse.bass as bass
import concourse.tile as tile
from concourse import bacc
from concourse import mybir
from concourse.bass_utils import run_bass_kernel_spmd

B, N, D = 32, 577, 768
NH, DH = 12, 64
NCORES = 8
BL = B // NCORES            # 4 batch elements per core
SCALE = DH ** -0.5
SX, SW = 4.0, 64.0            # fp8 hi/lo scales for x and W_qkv
SQ = 8.0                      # fp8 scale for the stored q8/k8
NP = 608                      # x8 padded row length (4B-aligned subrows)
SO, SWP = 64.0, 64.0          # fp8 hi/lo scales for O and W_proj
KT = D // 128               # 6 contraction tiles over D
TT = (N + 127) // 128       # 5 token tiles (4x128 + 65)
BF16 = np.float16

F32 = mybir.dt.float32
BF = mybir.dt.float16
F8 = mybir.dt.float8e4
DR = mybir.MatmulPerfMode.DoubleRow

def tok_m(t):
    return min(128, N - 128 * t)


def _build_nc():
    nc = bacc.Bacc()

    x8h_d = nc.declare_dram_parameter("x8h", [BL, 128, KT, NP], F8, isOutput=False)
    x8l_d = nc.declare_dram_parameter("x8l", [BL, 128, KT, NP], F8, isOutput=False)
    w8h_d = nc.declare_dram_parameter("w8h", [128, KT, 3 * D], F8, isOutput=False)
    w8l_d = nc.declare_dram_parameter("w8l", [128, KT, 3 * D], F8, isOutput=False)
    wp8h_d = nc.declare_dram_parameter("wp8h", [128, KT, D], F8, isOutput=False)
    wp8l_d = nc.declare_dram_parameter("wp8l", [128, KT, D], F8, isOutput=False)
    biasT_d = nc.declare_dram_parameter("biasT", [128, NH, TT, N], BF, isOutput=False)
    qkvb_d = nc.declare_dram_parameter("qkvb", [128, 12], F32, isOutput=False)
    vb_d = nc.declare_dram_parameter("vb", [1, D], BF, isOutput=False)
    pb_d = nc.declare_dram_parameter("pb", [1, D], BF, isOutput=False)
    out_d = nc.declare_dram_parameter("out", [BL, N, D], BF, isOutput=True)

    Exp = mybir.ActivationFunctionType.Exp
    MT_ORDER = [t for i in range(KT) for t in (i, KT + i)]
    QCH = [(0, 512), (512, N - 512)]          # free-dim chunks over 577
    DCH = [(0, 512), (512, D - 512)]          # free-dim chunks over 768
    QCH8 = [(0, 256), (256, 256), (512, N - 512)]    # DR moving <= 2*256
    DCH8 = [(0, 256), (256, 256), (512, 256)]
    ALPHA_QK = SQ / (SX * SW)   # psum -> q8/k8 scale

    with tile.TileContext(nc) as tc:
        with (
            tc.tile_pool(name="singles", bufs=1) as singles,
            tc.tile_pool(name="xt", bufs=2) as xt_pool,
            tc.tile_pool(name="qk8p", bufs=2) as qk8_pool,
            tc.tile_pool(name="vbuf", bufs=2) as v_pool,
            tc.tile_pool(name="exps", bufs=18) as exps_pool,
            tc.tile_pool(name="obuf", bufs=2) as o_pool,
            tc.tile_pool(name="otb", bufs=1) as ot_pool,
            tc.tile_pool(name="outs", bufs=2) as out_pool,
            tc.tile_pool(name="small", bufs=2) as small_pool,
            tc.tile_pool(name="ps512", bufs=2, space="PSUM") as ps512,
            tc.tile_pool(name="ps128", bufs=2, space="PSUM") as ps128,
            tc.tile_pool(name="psS", bufs=2, space="PSUM") as psS_pool,
        ):
            state = {}

            def make_qkv_units(b, split_x=False):
                """Per-tile qkv closures for batch b (used as PE fillers).

                fp8 hi/lo DoubleRow: psum accumulates 9 DR matmuls per tile
                (3 kt-pairs x {hh, hl, lh}), worth 4.5 f16 contraction steps
                instead of 6. q/k tiles drain to single-fp8 qk8 (x SQ);
                v drains to bf16 v_sb with the SX*SW/SO rowsum column.
                """
                x8h = xt_pool.tile([128, KT, NP], F8, name="x8h", tag="x8h")
                x8l = xt_pool.tile([128, KT, NP], F8, name="x8l", tag="x8l")
                if split_x:
                    state["x0"] = (x8h, x8l)
                    nc.sync.dma_start(out=x8h[:, 0:2, :], in_=x8h_d[b, :, 0:2, :])
                    nc.sync.dma_start(out=x8l[:, 0:2, :], in_=x8l_d[b, :, 0:2, :])
                else:
                    nc.sync.dma_start(out=x8h, in_=x8h_d[b])
                    nc.sync.dma_start(out=x8l, in_=x8l_d[b])
                qk8 = qk8_pool.tile([128, 13, NP], F8, name="qk8", tag="qk8")
                nc.vector.memset(qk8[:, 12, :], 0.0)   # shared DR zero subrow
                v_sb = v_pool.tile([128, TT, NH * 65], BF, name="v", tag="v")
                v_str = v_sb.rearrange("p t (h c) -> p t h c", c=65)
                state[b] = (qk8, v_sb)
                TERMS = ((0, 0), (1, 0), (0, 1))     # (x hi/lo, w hi/lo)

                def qk_phase(mt, ph, st):
                    if ph == 0:
                        st["pss"] = [
                            ps512.tile([128, 512], F32, name="ps_qk0", tag="a"),
                            ps128.tile([128, 128], F32, name="ps_qk1", tag="b")]
                    pss = st["pss"]
                    xi, wi = TERMS[ph]
                    xa = x8l if xi else x8h
                    wa = w8l if wi else w8h
                    for kp in range(KT // 2):
                        n = 3 * ph + kp + 1
                        for ci, (c0, w) in enumerate(QCH8):
                            nc.tensor.matmul(
                                pss[0][:, c0:c0 + w] if ci < 2
                                else pss[1][:, :w],
                                wa[:, 2 * kp:2 * kp + 2,
                                   128 * mt:128 * (mt + 1)],
                                xa[:, 2 * kp:2 * kp + 2, c0:c0 + w],
                                start=(n == 1 and ci in (0, 2)),
                                stop=(n == 9 and ci in (1, 2)),
                                perf_mode=DR,
                            )
                    if ph == 2:
                        for ci, (c0, w) in enumerate(QCH):
                            nc.vector.scalar_tensor_tensor(
                                qk8[:, mt, c0:c0 + w], pss[ci][:, :w],
                                ALPHA_QK,
                                qkvb[:, mt:mt + 1].to_broadcast([128, w]),
                                op0=mybir.AluOpType.mult,
                                op1=mybir.AluOpType.add,
                            )

                def v_phase(tt, ph, st):
                    m = tok_m(tt)
                    if ph == 0:
                        if tt == 0:
                            nc.vector.memset(v_str[:, :, :, 64:65],
                                             SX * SW / SO)
                        st["pss"] = [
                            ps512.tile([128, 512], F32, name="ps_v0", tag="a"),
                            ps512.tile([128, 512], F32, name="ps_v1", tag="a")]
                    pss = st["pss"]
                    xi, wi = TERMS[ph]
                    xa = x8l if xi else x8h
                    wa = w8l if wi else w8h
                    for kp in range(KT // 2):
                        n = 3 * ph + kp + 1
                        for ci, (c0, w) in enumerate(DCH8):
                            nc.tensor.matmul(
                                pss[0][:m, c0:c0 + w] if c0 < 512
                                else pss[1][:m, c0 - 512:c0 - 512 + w],
                                xa[:, 2 * kp:2 * kp + 2,
                                   128 * tt:128 * tt + m],
                                wa[:, 2 * kp:2 * kp + 2,
                                   2 * D + c0:2 * D + c0 + w],
                                start=(n == 1 and ci in (0, 2)),
                                stop=(n == 9 and ci in (1, 2)),
                                perf_mode=DR,
                            )
                    if ph == 2:
                        for ci, (c0, w) in enumerate(DCH):
                            nh0, nh1 = c0 // 64, (c0 + w) // 64
                            src_ps = (pss[ci][:m, :w] if ci == 0
                                      else pss[1][:m, :w])
                            nc.vector.tensor_add(
                                v_str[:m, tt, nh0:nh1, 0:64],
                                src_ps.rearrange("p (h c) -> p h c", c=64),
                                vbias[:m, c0:c0 + w].rearrange(
                                    "p (h c) -> p h c", c=64),
                            )

                def unit_phases(fn, key):
                    st = {}
                    return [lambda ph=ph: fn(key, ph, st) for ph in range(3)]

                qk_units = [p for mt in MT_ORDER
                            for p in unit_phases(qk_phase, mt)]
                v_units = [p for tt in range(TT)
                           for p in unit_phases(v_phase, tt)]
                return {"early": qk_units[0:18], "later": qk_units[18:36],
                        "v": v_units}

            def emit_scores(b, h, fill, mul_alternate=False):
                qk8, _ = state[b]
                po = 64 * (h % 2)
                jq, jk = h // 2, 6 + h // 2
                expS = [exps_pool.tile([128, N], BF, name="expS", tag="es")
                        for _ in range(TT)]
                for kt in range(TT):
                    km = tok_m(kt)
                    ps_s = psS_pool.tile([128, 768], F32, name="ps_s")
                    for ci, (c0, w) in enumerate(QCH8):
                        nc.tensor.matmul(
                            ps_s[:km, c0:c0 + w],
                            qk8[po:po + 64, jk:13:12 - jk,
                                128 * kt:128 * kt + km],
                            qk8[po:po + 64, jq:13:12 - jq, c0:c0 + w],
                            start=(ci in (0, 2)), stop=(ci in (1, 2)),
                            perf_mode=DR,
                        )
                    nc.scalar.activation(expS[kt][:km, :], ps_s[:km, :N],
                                         Exp, scale=SCALE / (SQ * SQ))
                    # exp(rel_bias) multiply, host-precomputed; mostly on the
                    # otherwise idle GpSimd (DVE for the very last head, so
                    # the tail P@V is not gated behind Pool's queue)
                    mul_eng = (nc.gpsimd if not mul_alternate or kt % 2 == 0
                               else nc.vector)
                    mul_eng.tensor_mul(
                        expS[kt][:km, :], expS[kt][:km, :],
                        biasT[:km, h, kt, :],
                    )
                    fill()
                return expS

            def emit_pav(b, h, expS):
                _, v_sb = state[b]
                o_sb = state[b, "o"]
                for qt in range(TT):
                    qm = tok_m(qt)
                    ps_o = ps128.tile([128, 128], F32, name="ps_o", tag="b")
                    for kt in range(TT):
                        km = tok_m(kt)
                        nc.tensor.matmul(
                            ps_o[:qm, :65],
                            expS[kt][:km, 128 * qt:128 * qt + qm],
                            v_sb[:km, kt, 65 * h:65 * h + 65],
                            start=(kt == 0), stop=(kt == TT - 1),
                        )
                    rcp = small_pool.tile([128, 1], F32, name="rcp", tag="rcp")
                    nc.vector.reciprocal(rcp[:qm], ps_o[:qm, 64:65])
                    nc.vector.tensor_mul(
                        o_sb[:qm, qt, 64 * h:64 * h + 64],
                        ps_o[:qm, 0:64],
                        rcp[:qm, 0:1].to_broadcast([qm, 64]),
                    )

            def make_proj_units(b):
                """DMA transposes + hi/lo split + fp8-DR output projection.

                o_sb holds SO*O (via the ones column); the transposed oT f16
                is split per token-tile into fp8 hi+lo on GpSimd, then the
                projection runs as 9 DoubleRow matmuls per tile. The
                SO*SWP descale fuses into the scalar_tensor_tensor copy.
                """
                o_sb = state[b, "o"]
                oT = ot_pool.tile([128, KT, TT, 128], BF, name="oT", tag="oT")
                oT8h = ot_pool.tile([128, KT, TT, 128], F8, name="oT8h",
                                    tag="oT8h")
                oT8l = ot_pool.tile([128, KT, TT, 128], F8, name="oT8l",
                                    tag="oT8l")
                for qt in range(TT):
                    nc.sync.dma_start_transpose(
                        oT[:, :, qt, :], o_sb[:, qt, :])

                def split(tt):
                    nc.gpsimd.tensor_copy(oT8h[:, :, tt, :], oT[:, :, tt, :])
                    nc.gpsimd.tensor_sub(
                        oT8l[:, :, tt, :], oT[:, :, tt, :], oT8h[:, :, tt, :])
                split(0)
                split(1)

                PTERMS = ((oT8h, wp8h), (oT8h, wp8l), (oT8l, wp8h))

                def proj_phase(tt, ph, st):
                    m = tok_m(tt)
                    if ph == 0:
                        if tt + 2 < TT:
                            split(tt + 2)
                        st["pss"] = [
                            ps512.tile([128, 512], F32, name="ps_p0", tag="a"),
                            ps512.tile([128, 512], F32, name="ps_p1", tag="a")]
                    pss = st["pss"]
                    oa, wa = PTERMS[ph]
                    for kp in range(KT // 2):
                        n = 3 * ph + kp + 1
                        for ci, (c0, w) in enumerate(DCH8):
                            nc.tensor.matmul(
                                pss[0][:m, c0:c0 + w] if c0 < 512
                                else pss[1][:m, c0 - 512:c0 - 512 + w],
                                oa[:, 2 * kp:2 * kp + 2, tt, :m],
                                wa[:, 2 * kp:2 * kp + 2, c0:c0 + w],
                                start=(n == 1 and ci in (0, 2)),
                                stop=(n == 9 and ci in (1, 2)),
                                perf_mode=DR,
                            )
                    if ph == 2:
                        out_sb = out_pool.tile([128, D], BF, name="out",
                                               tag="out")
                        for ci, (c0, w) in enumerate(DCH):
                            src_ps = (pss[ci][:m, :w] if ci == 0
                                      else pss[1][:m, :w])
                            nc.vector.scalar_tensor_tensor(
                                out_sb[:m, c0:c0 + w], src_ps,
                                1.0 / (SO * SWP), pbias[:m, c0:c0 + w],
                                op0=mybir.AluOpType.mult,
                                op1=mybir.AluOpType.add,
                            )
                            nc.sync.dma_start(
                                out=out_d[b, 128 * tt:128 * tt + m,
                                          c0:c0 + w],
                                in_=out_sb[:m, c0:c0 + w],
                            )

                def unit_phases(tt):
                    st = {}
                    return [lambda ph=ph: proj_phase(tt, ph, st)
                            for ph in range(3)]

                return [p for tt in range(TT) for p in unit_phases(tt)]

            def emit_tail(b, h, expS):
                """Last batch, last head. Heads 0-9 columns of o_sb are
                final already: transpose+split slots 0-4 up front, run the
                heads-0-7 portion of the projection for three token tiles
                (psums: one "a" pair + two idle 768-wide scores tiles), then
                per-qt P@V of the last head -> slot-5 transpose/split -> the
                remaining projection matmuls and drains."""
                _, v_sb = state[b]
                o_sb = state[b, "o"]
                oT = ot_pool.tile([128, KT, TT, 128], BF, name="oT", tag="oT")
                oT8h = ot_pool.tile([128, KT, TT, 128], F8, name="oT8h",
                                    tag="oT8h")
                oT8l = ot_pool.tile([128, KT, TT, 128], F8, name="oT8l",
                                    tag="oT8l")
                for qt in range(TT):
                    nc.sync.dma_start_transpose(
                        oT[:, 0:5, qt, :], o_sb[:, qt, 0:640])
                for qt in range(TT):
                    eng = nc.gpsimd if qt % 2 == 0 else nc.vector
                    eng.tensor_copy(oT8h[:, 0:5, qt, :], oT[:, 0:5, qt, :])
                    eng.tensor_sub(oT8l[:, 0:5, qt, :], oT[:, 0:5, qt, :],
                                   oT8h[:, 0:5, qt, :])

                TERMS3 = ((oT8h, wp8h), (oT8h, wp8l), (oT8l, wp8h))
                pss_of = {}
                nn = {}

                def alloc_ps(qt):
                    if qt in (0, 3):
                        pss_of[qt] = [
                            ps512.tile([128, 512], F32, name="ps_p0", tag="a"),
                            ps512.tile([128, 512], F32, name="ps_p1", tag="a")]
                    else:
                        pss_of[qt] = psS_pool.tile([128, 768], F32,
                                                   name="ps_s")
                    nn[qt] = 0

                def mm(qt, kps):
                    m = tok_m(qt)
                    ps = pss_of[qt]
                    for kp in kps:
                        for oa, wa in TERMS3:
                            nn[qt] += 1
                            n = nn[qt]
                            for ci, (c0, w) in enumerate(DCH8):
                                if isinstance(ps, list):
                                    out_ap = (ps[0][:m, c0:c0 + w] if c0 < 512
                                              else ps[1][:m,
                                                         c0 - 512:c0 - 512 + w])
                                else:
                                    out_ap = ps[:m, c0:c0 + w]
                                nc.tensor.matmul(
                                    out_ap,
                                    oa[:, 2 * kp:2 * kp + 2, qt, :m],
                                    wa[:, 2 * kp:2 * kp + 2, c0:c0 + w],
                                    start=(n == 1 and ci in (0, 2)),
                                    stop=(n == 9 and ci in (1, 2)),
                                    perf_mode=DR,
                                )

                def drain(qt, last=False):
                    m = tok_m(qt)
                    ps = pss_of[qt]
                    out_sb = out_pool.tile([128, D], BF, name="out", tag="out")
                    dch = DCH if not last else [(0, 256), (256, 256),
                                               (512, 256)]
                    for di, (c0, w) in enumerate(dch):
                        if isinstance(ps, list):
                            src_ps = (ps[0][:m, c0:c0 + w] if c0 < 512
                                      else ps[1][:m, c0 - 512:c0 - 512 + w])
                        else:
                            src_ps = ps[:m, c0:c0 + w]
                        nc.vector.scalar_tensor_tensor(
                            out_sb[:m, c0:c0 + w], src_ps,
                            1.0 / (SO * SWP), pbias[:m, c0:c0 + w],
                            op0=mybir.AluOpType.mult,
                            op1=mybir.AluOpType.add,
                        )
                        dq = ((nc.sync, nc.scalar, nc.gpsimd)[di % 3]
                              if last else nc.sync)
                        dq.dma_start(
                            out=out_d[b, 128 * qt:128 * qt + m, c0:c0 + w],
                            in_=out_sb[:m, c0:c0 + w],
                        )

                # heads-0-7 projection portion for units 0-2
                for qt in (0, 1, 2):
                    alloc_ps(qt)
                    mm(qt, (0, 1))
                # last head's P@V, normalize, slot-5 transpose+split
                for qt in range(TT):
                    qm = tok_m(qt)
                    ps_o = ps128.tile([128, 128], F32, name="ps_o", tag="b")
                    for kt in range(TT):
                        km = tok_m(kt)
                        nc.tensor.matmul(
                            ps_o[:qm, :65],
                            expS[kt][:km, 128 * qt:128 * qt + qm],
                            v_sb[:km, kt, 65 * h:65 * h + 65],
                            start=(kt == 0), stop=(kt == TT - 1),
                        )
                    rcp = small_pool.tile([128, 1], F32, name="rcp", tag="rcp")
                    nc.vector.reciprocal(rcp[:qm], ps_o[:qm, 64:65])
                    nc.vector.tensor_mul(
                        o_sb[:qm, qt, 64 * h:64 * h + 64],
                        ps_o[:qm, 0:64],
                        rcp[:qm, 0:1].to_broadcast([qm, 64]),
                    )
                    nc.sync.dma_start_transpose(
                        oT[:, 5, qt, :], o_sb[:, qt, 640:768])
                    eng = nc.gpsimd if qt % 2 == 0 else nc.vector
                    eng.tensor_copy(oT8h[:, 5, qt, :], oT[:, 5, qt, :])
                    eng.tensor_sub(oT8l[:, 5, qt, :], oT[:, 5, qt, :],
                                   oT8h[:, 5, qt, :])
                # finish units 0-2, then run 3-4 whole
                for qt in (0, 1, 2):
                    mm(qt, (2,))
                    drain(qt)
                alloc_ps(3)
                mm(3, (0, 1, 2))
                alloc_ps(4)
                mm(4, (0, 1))
                drain(3)
                mm(4, (2,))
                drain(4, last=True)

            # ---- one-time loads, split across the SP / Act / Pool DMA
            # queues so the first scores+exp can start ~10us in: SP carries
            # w8h-q interleaved with x8h[0], Act carries w8l-q (it is idle
            # until the first exp), Pool carries x8l[0] via SWDGE. ----
            qkvb = singles.tile([128, 12], F32)
            w8h = singles.tile([128, KT, 3 * D], F8)
            w8l = singles.tile([128, KT, 3 * D], F8)
            biasT = singles.tile([128, NH, TT, N], BF)
            first_units = None
            for kp in range(KT // 2):
                k0 = 2 * kp
                nc.sync.dma_start(out=w8h[:, k0:k0 + 2, 0:D],
                                  in_=w8h_d[:, k0:k0 + 2, 0:D])
                nc.scalar.dma_start(out=w8l[:, k0:k0 + 2, 0:D],
                                    in_=w8l_d[:, k0:k0 + 2, 0:D])
                if first_units is None:
                    first_units = make_qkv_units(0, split_x=True)
                    x8h0, x8l0 = state["x0"]
                    nc.gpsimd.dma_start(out=x8l0, in_=x8l_d[0])
                    # k-block of W via Pool SWDGE so scores(0,0) can start
                    # as soon as the q0/k0 tiles are computed
                    nc.gpsimd.dma_start(out=w8h[:, :, D:2 * D],
                                        in_=w8h_d[:, :, D:2 * D])
                    nc.gpsimd.dma_start(out=w8l[:, :, D:2 * D],
                                        in_=w8l_d[:, :, D:2 * D])
                else:
                    if k0 == 2:
                        nc.sync.dma_start(out=qkvb, in_=qkvb_d[:])
                    nc.sync.dma_start(out=x8h0[:, k0:k0 + 2, :],
                                      in_=x8h_d[0, :, k0:k0 + 2, :])
            nc.scalar.dma_start(out=biasT[:, 0], in_=biasT_d[:, 0])
            nc.sync.dma_start(out=biasT[:, 1], in_=biasT_d[:, 1])
            nc.sync.dma_start(out=w8h[:, :, 2 * D:], in_=w8h_d[:, :, 2 * D:])
            nc.gpsimd.dma_start(out=w8l[:, :, 2 * D:], in_=w8l_d[:, :, 2 * D:])
            vbias = singles.tile([128, D], BF)
            nc.sync.dma_start(out=vbias, in_=vb_d[:].to_broadcast([128, D]))
            wp8h = singles.tile([128, KT, D], F8)
            nc.sync.dma_start(out=wp8h, in_=wp8h_d[:])
            wp8l = singles.tile([128, KT, D], F8)
            nc.gpsimd.dma_start(out=wp8l, in_=wp8l_d[:])
            pbias = singles.tile([128, D], BF)
            nc.sync.dma_start(out=pbias, in_=pb_d[:].to_broadcast([128, D]))
            # rel-bias table: heads 2-3 via Pool SWDGE, the rest on SP in
            # head order (low heads first so early bias-muls are not gated).
            nc.gpsimd.dma_start(out=biasT[:, 2], in_=biasT_d[:, 2])
            nc.gpsimd.dma_start(out=biasT[:, 3], in_=biasT_d[:, 3])
            for h in range(4, NH):
                nc.sync.dma_start(out=biasT[:, h], in_=biasT_d[:, h])

            # ---- schedule: one fused 48-head stream (batch boundaries
            # are software-pipelined away), PV lag 2, fillers paced so the
            # in-order PE queue always has ready work while Act/Pool chew
            # through exp / bias-mul of earlier score tiles. ----
            units = {0: first_units}
            for u in first_units["early"]:     # q0,k0,q1,k1,q2,k2
                u()
            fillers = []
            base = {}
            done = [0]
            slot = [0]
            win = [0, 1]     # [window start slot, window filler end]
            first_win = [True]

            def fill(need=None):
                slot[0] += 1
                frac = (slot[0] - win[0]) / 71.0
                due = int(win[1] + frac * (len(fillers) - win[1]))
                due = min(len(fillers), due, done[0] + 2)
                if need is not None:
                    due = max(due, min(need, len(fillers)))
                while done[0] < due:
                    fillers[done[0]]()
                    done[0] += 1

            def alloc_o(b):
                state[b, "o"] = o_pool.tile([128, TT, D], BF, name="o", tag="o")
                # rows past the last token tile are read by the transpose
                # (and masked downstream); keep them initialized
                nc.vector.memset(state[b, "o"][64:, TT - 1, :], 0.0)

            pend = []
            for b in range(BL):
                base[b] = len(fillers)
                win[0] = slot[0]
                win[1] = done[0]
                first_win[0] = b == 0
                fillers.extend(units[b]["v"])
                fillers.extend(units[b]["later"])
                if b + 1 < BL:
                    units[b + 1] = make_qkv_units(b + 1)
                    fillers.extend(units[b + 1]["early"])
                for h in range(NH):
                    if h == 2 and b > 0:
                        # o_sb(b-1) is complete only once PV(b-1,11) has
                        # been emitted (pend lag 2); create proj(b-1) now
                        # but slot it mid-window (after the later-qk block)
                        pos = max(done[0], base[b] + 33)
                        fillers[pos:pos] = make_proj_units(b - 1)
                    if 5 <= h <= 10:
                        # later q/k tiles land one per head ahead of their
                        # scores (avoids bursty catch-up stalls)
                        fill(need=base[b] + 3 * (h + 1))
                    pend.append((b, h, emit_scores(b, h, fill)))
                    if len(pend) > 2:
                        pb_, ph, pexp = pend.pop(0)
                        if ph == 0:
                            fill(need=base[pb_] + 15)  # v before P@V(b,0)
                            alloc_o(pb_)   # late: frees o-pool rotation
                        emit_pav(pb_, ph, pexp)
                    fill()
            # drain: PV(3,10), flush remaining fillers, then the pipelined
            # tail (PV(3,11) per-qt -> transpose -> split -> projection)
            pb_, ph, pexp = pend.pop(0)
            emit_pav(pb_, ph, pexp)
            while done[0] < len(fillers):
                fillers[done[0]]()
                done[0] += 1
            pb_, ph, pexp = pend.pop(0)
            emit_tail(pb_, ph, pexp)
    nc.finalize()
    return nc


_NC_CACHE = {}


def _get_nc():
    if "nc" not in _NC_CACHE:
        _NC_CACHE["nc"] = _build_nc()
    return _NC_CACHE["nc"]


def _prep_shared(qkv_w, q_bias, v_bias, rpb_table, proj_w, proj_b, rel_index):
    import ml_dtypes
    E4M3 = ml_dtypes.float8_e4m3
    qkv_w = np.asarray(qkv_w, dtype=np.float32)
    qkv_bias_full = np.concatenate([
        np.asarray(q_bias, np.float32),
        np.zeros(D, np.float32),
        np.asarray(v_bias, np.float32),
    ])
    # fp8 hi/lo split of SW*W in the [128, KT, 3D] device layout
    wt = np.ascontiguousarray(
        qkv_w.T.reshape(KT, 128, 3 * D).transpose(1, 0, 2)) * SW
    w8h = wt.astype(E4M3)
    w8l = (wt - w8h.astype(np.float32)).astype(E4M3)
    # q/k bias at the SQ-scaled fp8 level
    qkvb = np.ascontiguousarray(
        (qkv_bias_full[:2 * D] * SQ).reshape(12, 128).T).astype(np.float32)
    wpt = np.ascontiguousarray(
        np.asarray(proj_w, np.float32).T.reshape(KT, 128, D)
        .transpose(1, 0, 2)) * SWP
    wp8h = wpt.astype(E4M3)
    wp8l = (wpt - wp8h.astype(np.float32)).astype(E4M3)
    # relative position bias, transposed to [k, q] and padded to 640 rows
    rb = np.asarray(rpb_table, np.float32)[
        np.asarray(rel_index).reshape(-1)].reshape(N, N, NH)  # [q, k, h]
    rbp = np.zeros((TT * 128, N, NH), np.float32)
    rbp[:N] = rb.transpose(1, 0, 2)                            # [k, q, h]
    biasT = np.ascontiguousarray(
        np.exp(rbp.reshape(TT, 128, N, NH).transpose(1, 3, 0, 2))).astype(BF16)
    vb = np.ascontiguousarray(
        (qkv_bias_full[2 * D:] * (SX * SW)).reshape(1, D)).astype(BF16)
    pb = np.ascontiguousarray(
        np.asarray(proj_b, np.float32).reshape(1, D)).astype(BF16)
    return w8h, w8l, wp8h, wp8l, qkvb, biasT, vb, pb


def _make_in_maps(inputs):
    import ml_dtypes
    E4M3 = ml_dtypes.float8_e4m3
    x = np.asarray(inputs["x"], dtype=np.float32)
    w8h, w8l, wp8h, wp8l, qkvb, biasT, vb, pb = _prep_shared(
        inputs["qkv_w"], inputs["q_bias"], inputs["v_bias"],
        inputs["rpb_table"], inputs["proj_w"], inputs["proj_b"],
        inputs["rel_index"])

    in_maps = []
    for i in range(NCORES):
        xs = x[i * BL:(i + 1) * BL]                            # [BL, N, D]
        xT = np.zeros((BL, 128, KT, NP), np.float32)
        xT[..., :N] = xs.transpose(0, 2, 1).reshape(BL, KT, 128, N)\
            .transpose(0, 2, 1, 3) * SX
        x8h = xT.astype(E4M3)
        x8l = (xT - x8h.astype(np.float32)).astype(E4M3)
        in_maps.append({
            "x8h": x8h, "x8l": x8l, "w8h": w8h, "w8l": w8l,
            "wp8h": wp8h, "wp8l": wp8l, "biasT": biasT,
            "qkvb": qkvb, "vb": vb, "pb": pb,
        })

    return in_maps


def kernel(**inputs):
    in_maps = _make_in_maps(inputs)
    nc = _get_nc()
    res = run_bass_kernel_spmd(nc, in_maps, core_ids=list(range(NCORES)))
    out = np.concatenate([res.results[i]["out"] for i in range(NCORES)], axis=0)
    return np.ascontiguousarray(out.astype(np.float32))


def kernel_traced(**inputs):
    """Like kernel() but also returns (out, BassKernelResults with profile)."""
    in_maps = _make_in_maps(inputs)
    nc = _get_nc()
    res = run_bass_kernel_spmd(nc, in_maps, core_ids=list(range(NCORES)),
                               trace=True)
    out = np.concatenate([res.results[i]["out"] for i in range(NCORES)], axis=0)
    return np.ascontiguousarray(out.astype(np.float32)), res
